# revision 2
# baseline (speedup 1.0000x reference)
"""Sparse (block-local) attention for B=2, Sq=2048, Sk=4096, D=1024, H=16.

Each query i attends to exactly keys {2i, 2i+1} (Sk/Sq == 2, no remainder),
so softmax is over 2 scores -> p1 = sigmoid((s1-s2)*scale), p2 = sigmoid((s2-s1)*scale).

Distribution: sequence-parallel over (batch, query-block). 8 cores, each takes
512 contiguous queries of one batch plus the matching 1024 contiguous keys.
No collectives needed; outputs are concatenated on the host.

Per-core device kernel (all matmuls bf16 with fp32 PSUM accumulation):
  Q  = x_s  @ Wq^T           row-major   [512, 1024]
  K  = c_perm @ Wk^T         row-major   [1024, 1024] (keys permuted even|odd)
  V  = c_perm @ Wv^T         row-major   [1024, 1024]
  s1/s2 row-wise dots on DVE (mul + grouped reduce per 64-dim head)
  p1/p2 on ACT (sigmoid), AV combine on DVE -> att [512, 1024]
  att^T via PE transposes, O = att @ Wo^T, DMA out.

Host side only reshapes/shards/casts: transposes x/c/weights to feature-major,
permutes keys even|odd, casts to bf16, and concatenates the 8 core outputs.
"""

import sys

for _p in ("/opt/trn_rl_repo",):
    if _p not in sys.path:
        sys.path.append(_p)

import numpy as np
import ml_dtypes

import concourse.bass as bass
import concourse.mybir as mybir
import concourse.tile as tile
from concourse import bacc
from concourse.bass_utils import run_bass_kernel_spmd
from concourse.masks import make_identity

B, SQ, SK, D, H, HD = 2, 2048, 4096, 1024, 16, 64
N_CORES = 8
QL = B * SQ // N_CORES       # 512 queries per core
KL = 2 * QL                  # 1024 keys per core
QT = QL // 128               # 4 query tiles
NB = 512                     # matmul moving free dim / psum bank width (fp32)
JT = D // NB                 # 2 output-column blocks per projection
SCALE = 1.0 / float(np.sqrt(HD))

FB = mybir.dt.bfloat16
F32 = mybir.dt.float32
BF = ml_dtypes.bfloat16


def _build(kd_tiles: int, with_bo: bool):
    """Build + finalize the per-core Bacc graph (SPMD: same graph on 8 cores)."""
    KD = kd_tiles * 128
    nc = bacc.Bacc("TRN2", target_bir_lowering=False)

    xT = nc.dram_tensor("xT", [KD, QL], FB, kind="ExternalInput")
    cT = nc.dram_tensor("cT", [KD, KL], FB, kind="ExternalInput")
    wq = nc.dram_tensor("wq", [KD, D], FB, kind="ExternalInput")
    wk = nc.dram_tensor("wk", [KD, D], FB, kind="ExternalInput")
    wv = nc.dram_tensor("wv", [KD, D], FB, kind="ExternalInput")
    wo = nc.dram_tensor("wo", [D, D], FB, kind="ExternalInput")
    bo = None
    if with_bo:
        bo = nc.dram_tensor("bo", [1, D], F32, kind="ExternalInput")
    out = nc.dram_tensor("out", [QL, D], F32, kind="ExternalOutput")

    with tile.TileContext(nc) as tc:
        with (
            tc.tile_pool(name="ins", bufs=1) as ins,
            tc.tile_pool(name="acts", bufs=1) as acts,
            tc.tile_pool(name="att", bufs=4) as att,
            tc.tile_pool(name="outs", bufs=4) as outs,
            tc.tile_pool(name="psum", bufs=6, space="PSUM") as psum,
            tc.tile_pool(name="psum_tr", bufs=2, space="PSUM") as psum_tr,
        ):
            # ---- inputs to SBUF -------------------------------------------
            xT_sb = ins.tile([128, kd_tiles, QL], FB)
            cT_sb = ins.tile([128, kd_tiles, KL], FB)
            wq_sb = ins.tile([128, kd_tiles, D], FB)
            wk_sb = ins.tile([128, kd_tiles, D], FB)
            wv_sb = ins.tile([128, kd_tiles, D], FB)
            wo_sb = ins.tile([128, D // 128, D], FB)
            ident = ins.tile([128, 128], FB)
            make_identity(nc, ident)

            nc.sync.dma_start(out=xT_sb, in_=xT.rearrange("(t p) q -> p t q", p=128))
            nc.sync.dma_start(out=wq_sb, in_=wq.rearrange("(t p) j -> p t j", p=128))
            nc.sync.dma_start(out=cT_sb, in_=cT.rearrange("(t p) q -> p t q", p=128))
            nc.sync.dma_start(out=wk_sb, in_=wk.rearrange("(t p) j -> p t j", p=128))
            nc.sync.dma_start(out=wv_sb, in_=wv.rearrange("(t p) j -> p t j", p=128))
            nc.sync.dma_start(out=wo_sb, in_=wo.rearrange("(t p) j -> p t j", p=128))
            bo_sb = None
            if with_bo:
                bo_sb = ins.tile([128, D], F32)
                nc.sync.dma_start(out=bo_sb, in_=bo.to_broadcast((128, D)))

            # psum -> sbuf copy engine alternation (DVE / ACT load balance)
            copy_flip = [0]

            def pcopy(dst, src):
                if copy_flip[0] % 2 == 0:
                    nc.vector.tensor_copy(dst, src)
                else:
                    nc.scalar.copy(dst, src)
                copy_flip[0] += 1

            # ---- projections ----------------------------------------------
            q_sb = acts.tile([128, QT, D], FB)           # Q row-major
            k_sb = acts.tile([128, 2 * QT, D], FB)       # rows 0..3 even, 4..7 odd
            v_sb = acts.tile([128, 2 * QT, D], FB)

            def proj(dst_tile, dst_idx, act_sb, col0, w_sb):
                for jb in range(JT):
                    ps = psum.tile([128, NB], F32, tag="mm")
                    for kd in range(kd_tiles):
                        nc.tensor.matmul(
                            ps,
                            lhsT=act_sb[:, kd, col0:col0 + 128],
                            rhs=w_sb[:, kd, jb * NB:(jb + 1) * NB],
                            start=(kd == 0),
                            stop=(kd == kd_tiles - 1),
                        )
                    pcopy(dst_tile[:, dst_idx, jb * NB:(jb + 1) * NB], ps)

            for qt in range(QT):
                proj(q_sb, qt, xT_sb, qt * 128, wq_sb)
            for qt in range(QT):
                # even keys tile then odd keys tile for this query tile
                proj(k_sb, qt, cT_sb, qt * 128, wk_sb)
                proj(k_sb, QT + qt, cT_sb, QL + qt * 128, wk_sb)
            for qt in range(QT):
                proj(v_sb, qt, cT_sb, qt * 128, wv_sb)
                proj(v_sb, QT + qt, cT_sb, QL + qt * 128, wv_sb)

            # ---- attention (per query tile) -------------------------------
            av_sb = acts.tile([128, QT, D], FB)          # p1*V_even + p2*V_odd
            for qt in range(QT):
                qv = q_sb[:, qt, :]
                ke = k_sb[:, qt, :]
                ko = k_sb[:, QT + qt, :]
                pe = att.tile([128, H, HD], F32, tag="prod")
                po = att.tile([128, H, HD], F32, tag="prod")
                nc.vector.tensor_mul(pe.rearrange("p h e -> p (h e)"), qv, ke)
                nc.vector.tensor_mul(po.rearrange("p h e -> p (h e)"), qv, ko)
                s1 = att.tile([128, H], F32, tag="s")
                s2 = att.tile([128, H], F32, tag="s")
                nc.vector.reduce_sum(out=s1, in_=pe, axis=mybir.AxisListType.X)
                nc.vector.reduce_sum(out=s2, in_=po, axis=mybir.AxisListType.X)
                d12 = att.tile([128, H], F32, tag="s")
                nc.vector.tensor_sub(d12, s1, s2)
                p1 = att.tile([128, H], F32, tag="s")
                p2 = att.tile([128, H], F32, tag="s")
                nc.scalar.activation(p1, d12, mybir.ActivationFunctionType.Sigmoid,
                                     scale=SCALE)
                nc.scalar.activation(p2, d12, mybir.ActivationFunctionType.Sigmoid,
                                     scale=-SCALE)
                t1 = att.tile([128, H, HD], F32, tag="prod")
                t2 = att.tile([128, H, HD], F32, tag="prod")
                ve = v_sb[:, qt, :].rearrange("p (h e) -> p h e", h=H)
                vo = v_sb[:, QT + qt, :].rearrange("p (h e) -> p h e", h=H)
                nc.vector.tensor_mul(t1, ve, p1.to_broadcast((128, H, HD)))
                nc.vector.tensor_mul(t2, vo, p2.to_broadcast((128, H, HD)))
                nc.vector.tensor_add(
                    av_sb[:, qt, :],
                    t1.rearrange("p h e -> p (h e)"),
                    t2.rearrange("p h e -> p (h e)"),
                )

            # ---- transpose att -> attT, O projection, DMA out -------------
            avT_sb = acts.tile([128, D // 128, QL], FB)  # att^T feature-major
            for qt in range(QT):
                for db in range(D // 128):
                    tp = psum_tr.tile([128, 128], FB, tag="tr")
                    nc.tensor.transpose(tp, av_sb[:, qt, db * 128:(db + 1) * 128],
                                        ident)
                    pcopy(avT_sb[:, db, qt * 128:(qt + 1) * 128], tp)

            out_r = out.rearrange("(t p) j -> p t j", p=128)
            for qt in range(QT):
                for jb in range(JT):
                    ps = psum.tile([128, NB], F32, tag="mm")
                    for kd in range(D // 128):
                        nc.tensor.matmul(
                            ps,
                            lhsT=avT_sb[:, kd, qt * 128:(qt + 1) * 128],
                            rhs=wo_sb[:, kd, jb * NB:(jb + 1) * NB],
                            start=(kd == 0),
                            stop=(kd == D // 128 - 1),
                        )
                    o_t = outs.tile([128, NB], F32, tag="o")
                    if with_bo:
                        nc.vector.tensor_add(o_t, ps,
                                             bo_sb[:, jb * NB:(jb + 1) * NB])
                    else:
                        pcopy(o_t, ps)
                    nc.sync.dma_start(out=out_r[:, qt, jb * NB:(jb + 1) * NB],
                                      in_=o_t)

    nc.finalize()
    return nc


_GRAPH_CACHE = {}


def _get_graph(kd_tiles: int, with_bo: bool):
    key = (kd_tiles, with_bo)
    if key not in _GRAPH_CACHE:
        _GRAPH_CACHE[key] = _build(kd_tiles, with_bo)
    return _GRAPH_CACHE[key]


def _make_in_maps(x, c, Wq, bq, Wk, bk, Wv, bv, Wo, bo):
    x = np.asarray(x, np.float32)
    c = np.asarray(c, np.float32)
    has_bias = any(np.any(np.asarray(b)) for b in (bq, bk, bv))
    with_bo = bool(np.any(np.asarray(bo)))
    kd_tiles = D // 128 + (1 if has_bias else 0)
    KD = kd_tiles * 128

    def aug_w(W, b):
        wT = np.asarray(W, np.float32).T          # [D, D] feature-major
        if has_bias:
            pad = np.zeros((KD - D, D), np.float32)
            pad[0, :] = np.asarray(b, np.float32)
            wT = np.concatenate([wT, pad], axis=0)
        return np.ascontiguousarray(wT).astype(BF)

    wq_h = aug_w(Wq, bq)
    wk_h = aug_w(Wk, bk)
    wv_h = aug_w(Wv, bv)
    wo_h = np.ascontiguousarray(np.asarray(Wo, np.float32).T).astype(BF)

    def aug_act(aT):
        if has_bias:
            pad = np.zeros((KD - D, aT.shape[1]), np.float32)
            pad[0, :] = 1.0
            aT = np.concatenate([aT, pad], axis=0)
        return np.ascontiguousarray(aT).astype(BF)

    in_maps = []
    for core in range(N_CORES):
        b = core // (N_CORES // B)
        q0 = (core % (N_CORES // B)) * QL
        k0 = 2 * q0
        xs = x[b, q0:q0 + QL]                      # [QL, D]
        cs = c[b, k0:k0 + KL]                      # [KL, D]
        cperm = np.concatenate([cs[0::2], cs[1::2]], axis=0)
        m = {
            "xT": aug_act(xs.T),
            "cT": aug_act(cperm.T),
            "wq": wq_h,
            "wk": wk_h,
            "wv": wv_h,
            "wo": wo_h,
        }
        if with_bo:
            m["bo"] = np.asarray(bo, np.float32).reshape(1, D)
        in_maps.append(m)
    return in_maps, kd_tiles, with_bo


def _gather(results):
    out = np.empty((B, SQ, D), np.float32)
    for core in range(N_CORES):
        b = core // (N_CORES // B)
        q0 = (core % (N_CORES // B)) * QL
        out[b, q0:q0 + QL] = results[core]["out"]
    return out


def kernel(**inputs) -> np.ndarray:
    in_maps, kd_tiles, with_bo = _make_in_maps(**inputs)
    nc = _get_graph(kd_tiles, with_bo)
    res = run_bass_kernel_spmd(nc, in_maps, core_ids=list(range(N_CORES)))
    return _gather(res.results)


def run_traced(**inputs):
    """Like kernel() but with neuron-profile tracing; returns (out, results)."""
    in_maps, kd_tiles, with_bo = _make_in_maps(**inputs)
    nc = _get_graph(kd_tiles, with_bo)
    res = run_bass_kernel_spmd(nc, in_maps, core_ids=list(range(N_CORES)),
                               trace=True)
    return _gather(res.results), res


# revision 3
# speedup vs baseline: 1.1010x; 1.1010x over previous
"""Sparse (block-local) attention for B=2, Sq=2048, Sk=4096, D=1024, H=16.

Each query i attends to exactly keys {2i, 2i+1} (Sk/Sq == 2, no remainder),
so softmax is over 2 scores -> p1 = sigmoid((s1-s2)*scale), p2 = sigmoid((s2-s1)*scale).

Distribution: sequence-parallel over (batch, query-block). 8 cores, each takes
512 contiguous queries of one batch plus the matching 1024 contiguous keys.
No collectives needed; outputs are concatenated on the host.

Per-core device kernel (all matmuls bf16 with fp32 PSUM accumulation):
  Q  = x_s  @ Wq^T           row-major   [512, 1024]
  K  = c_perm @ Wk^T         row-major   [1024, 1024] (keys permuted even|odd)
  V  = c_perm @ Wv^T         row-major   [1024, 1024]
  s1/s2 row-wise dots on DVE (mul + grouped reduce per 64-dim head)
  p1/p2 on ACT (sigmoid), AV combine on DVE -> att [512, 1024]
  att^T via PE transposes, O = att @ Wo^T, DMA out.

Host side only reshapes/shards/casts: feature-major + partition-major tiled
layouts, keys permuted even|odd, cast to bf16, concatenate core outputs.

Engine budget: PE ~89us (the bottleneck), ACT does all projection-PSUM
copies so DVE is free to run attention as soon as its inputs land.
"""

import sys

for _p in ("/opt/trn_rl_repo",):
    if _p not in sys.path:
        sys.path.append(_p)

import numpy as np
import ml_dtypes

import concourse.bass as bass
import concourse.mybir as mybir
import concourse.tile as tile
from concourse import bacc
from concourse.bass_utils import run_bass_kernel_spmd
from concourse.masks import make_identity

B, SQ, SK, D, H, HD = 2, 2048, 4096, 1024, 16, 64
N_CORES = 8
QL = B * SQ // N_CORES       # 512 queries per core
KL = 2 * QL                  # 1024 keys per core
QT = QL // 128               # 4 query tiles
NB = 512                     # psum bank width (fp32)
JT = D // NB                 # 2 output-column blocks per projection
DT = D // 128                # 8 feature tiles
SCALE = 1.0 / float(np.sqrt(HD))

FB = mybir.dt.bfloat16
F32 = mybir.dt.float32
BF = ml_dtypes.bfloat16


def _build(kd_tiles: int, with_bo: bool):
    """Build + finalize the per-core Bacc graph (SPMD: same graph on 8 cores)."""
    nc = bacc.Bacc("TRN2", target_bir_lowering=False)
    ka = kd_tiles // 2          # first-half kd tiles (split DMA for early PE start)
    kb = kd_tiles - ka

    # All activation/weight inputs are host-arranged partition-major:
    # tensor[p, t, n] = logical[t*128 + p, n], so DMA descriptors are
    # per-partition contiguous.
    xT = nc.dram_tensor("xT", [128, kd_tiles, QL], FB, kind="ExternalInput")
    cT = nc.dram_tensor("cT", [128, kd_tiles, KL], FB, kind="ExternalInput")
    wq = nc.dram_tensor("wq", [128, kd_tiles, D], FB, kind="ExternalInput")
    wk = nc.dram_tensor("wk", [128, kd_tiles, D], FB, kind="ExternalInput")
    wv = nc.dram_tensor("wv", [128, kd_tiles, D], FB, kind="ExternalInput")
    wo = nc.dram_tensor("wo", [128, DT, D], FB, kind="ExternalInput")
    bo = None
    if with_bo:
        bo = nc.dram_tensor("bo", [1, D], F32, kind="ExternalInput")
    out = nc.dram_tensor("out", [128, QT, D], F32, kind="ExternalOutput")

    with tile.TileContext(nc) as tc:
        with (
            tc.tile_pool(name="ins", bufs=1) as ins,
            tc.tile_pool(name="acts", bufs=1) as acts,
            tc.tile_pool(name="att", bufs=4) as att,
            tc.tile_pool(name="outs", bufs=4) as outs,
            tc.tile_pool(name="psum", bufs=6, space="PSUM") as psum,
            tc.tile_pool(name="psum_tr", bufs=2, space="PSUM") as psum_tr,
        ):
            # ---- inputs to SBUF (order = order of first use) ---------------
            xa = ins.tile([128, ka, QL], FB)
            xb = ins.tile([128, kb, QL], FB)
            wqa = ins.tile([128, ka, D], FB)
            wqb = ins.tile([128, kb, D], FB)
            cT_sb = ins.tile([128, kd_tiles, KL], FB)
            wk_sb = ins.tile([128, kd_tiles, D], FB)
            wv_sb = ins.tile([128, kd_tiles, D], FB)
            wo_sb = ins.tile([128, DT, D], FB)
            ident = ins.tile([128, 128], FB)

            nc.sync.dma_start(out=xa, in_=xT[:, 0:ka, :])
            nc.sync.dma_start(out=wqa, in_=wq[:, 0:ka, :])
            nc.sync.dma_start(out=xb, in_=xT[:, ka:, :])
            nc.sync.dma_start(out=wqb, in_=wq[:, ka:, :])
            nc.sync.dma_start(out=cT_sb, in_=cT[:])
            nc.sync.dma_start(out=wk_sb, in_=wk[:])
            nc.sync.dma_start(out=wv_sb, in_=wv[:])
            nc.sync.dma_start(out=wo_sb, in_=wo[:])
            make_identity(nc, ident)
            bo_sb = None
            if with_bo:
                bo_sb = ins.tile([128, D], F32)
                nc.sync.dma_start(out=bo_sb, in_=bo.to_broadcast((128, D)))

            def x_slice(kd, col0):
                t, sb = (kd, xa) if kd < ka else (kd - ka, xb)
                return sb[:, t, col0:col0 + 128]

            def wq_slice(kd, jb):
                t, sb = (kd, wqa) if kd < ka else (kd - ka, wqb)
                return sb[:, t, jb * NB:(jb + 1) * NB]

            # ---- projections (psum copies all on ACT) ----------------------
            q_sb = acts.tile([128, QT, D], FB)           # Q row-major
            k_sb = acts.tile([128, 2 * QT, D], FB)       # rows 0..3 even, 4..7 odd
            v_sb = acts.tile([128, 2 * QT, D], FB)

            def mm_group(dst_tile, dst_idx, lhs_fn, rhs_fn):
                for jb in range(JT):
                    ps = psum.tile([128, NB], F32, tag="mm")
                    for kd in range(kd_tiles):
                        nc.tensor.matmul(
                            ps,
                            lhsT=lhs_fn(kd),
                            rhs=rhs_fn(kd, jb),
                            start=(kd == 0),
                            stop=(kd == kd_tiles - 1),
                        )
                    nc.scalar.copy(dst_tile[:, dst_idx, jb * NB:(jb + 1) * NB], ps)

            def proj_c(dst_tile, dst_idx, col0, w_sb):
                mm_group(
                    dst_tile, dst_idx,
                    lambda kd: cT_sb[:, kd, col0:col0 + 128],
                    lambda kd, jb: w_sb[:, kd, jb * NB:(jb + 1) * NB],
                )

            # attention state per query tile
            av_sb = acts.tile([128, QT, D], FB)          # p1*V_even + p2*V_odd

            def attention(qt):
                qv = q_sb[:, qt, :]
                ke = k_sb[:, qt, :]
                ko = k_sb[:, QT + qt, :]
                pe = att.tile([128, H, HD], FB, tag="prod")
                po = att.tile([128, H, HD], FB, tag="prod")
                nc.vector.tensor_mul(pe.rearrange("p h e -> p (h e)"), qv, ke)
                nc.vector.tensor_mul(po.rearrange("p h e -> p (h e)"), qv, ko)
                s1 = att.tile([128, H], F32, tag="s")
                s2 = att.tile([128, H], F32, tag="s")
                nc.vector.reduce_sum(out=s1, in_=pe, axis=mybir.AxisListType.X)
                nc.vector.reduce_sum(out=s2, in_=po, axis=mybir.AxisListType.X)
                d12 = att.tile([128, H], F32, tag="s")
                nc.vector.tensor_sub(d12, s1, s2)
                p1 = att.tile([128, H], F32, tag="s")
                p2 = att.tile([128, H], F32, tag="s")
                nc.scalar.activation(p1, d12, mybir.ActivationFunctionType.Sigmoid,
                                     scale=SCALE)
                nc.scalar.activation(p2, d12, mybir.ActivationFunctionType.Sigmoid,
                                     scale=-SCALE)
                t1 = att.tile([128, H, HD], F32, tag="prodf")
                t2 = att.tile([128, H, HD], F32, tag="prodf")
                ve = v_sb[:, qt, :].rearrange("p (h e) -> p h e", h=H)
                vo = v_sb[:, QT + qt, :].rearrange("p (h e) -> p h e", h=H)
                nc.vector.tensor_mul(t1, ve, p1.to_broadcast((128, H, HD)))
                nc.vector.tensor_mul(t2, vo, p2.to_broadcast((128, H, HD)))
                nc.vector.tensor_add(
                    av_sb[:, qt, :],
                    t1.rearrange("p h e -> p (h e)"),
                    t2.rearrange("p h e -> p (h e)"),
                )

            # Q for all query tiles first (only needs xT/wq DMA halves)
            for qt in range(QT):
                mm_group(q_sb, qt, lambda kd, qt=qt: x_slice(kd, qt * 128),
                         wq_slice)
            # K/V pairs per qt; attention(qt) emitted one qt later so its ACT
            # sigmoid never stalls the projection-copy stream
            for qt in range(QT):
                proj_c(k_sb, qt, qt * 128, wk_sb)
                proj_c(k_sb, QT + qt, QL + qt * 128, wk_sb)
                proj_c(v_sb, qt, qt * 128, wv_sb)
                proj_c(v_sb, QT + qt, QL + qt * 128, wv_sb)
                if qt >= 1:
                    attention(qt - 1)
            attention(QT - 1)

            # ---- transpose att -> attT (copies on ACT), O groups interleaved
            avT_sb = acts.tile([128, DT, QL], FB)        # att^T feature-major

            def transposes(qt):
                for db in range(DT):
                    tp = psum_tr.tile([128, 128], FB, tag="tr")
                    nc.tensor.transpose(tp, av_sb[:, qt, db * 128:(db + 1) * 128],
                                        ident)
                    nc.scalar.copy(avT_sb[:, db, qt * 128:(qt + 1) * 128], tp)

            def o_group(qt):
                for jb in range(JT):
                    ps = psum.tile([128, NB], F32, tag="mm")
                    for kd in range(DT):
                        nc.tensor.matmul(
                            ps,
                            lhsT=avT_sb[:, kd, qt * 128:(qt + 1) * 128],
                            rhs=wo_sb[:, kd, jb * NB:(jb + 1) * NB],
                            start=(kd == 0),
                            stop=(kd == DT - 1),
                        )
                    o_t = outs.tile([128, NB], F32, tag="o")
                    if with_bo:
                        nc.vector.tensor_add(o_t, ps,
                                             bo_sb[:, jb * NB:(jb + 1) * NB])
                    else:
                        nc.vector.tensor_copy(o_t, ps)
                    nc.sync.dma_start(out=out[:, qt, jb * NB:(jb + 1) * NB],
                                      in_=o_t)

            # PE order: T0 T1 O0 T2 O1 T3 O2 O3 — keeps PE fed while ACT
            # copies each avT tile group
            transposes(0)
            transposes(1)
            o_group(0)
            transposes(2)
            o_group(1)
            transposes(3)
            o_group(2)
            o_group(3)

    nc.finalize()
    return nc


_GRAPH_CACHE = {}


def _get_graph(kd_tiles: int, with_bo: bool):
    key = (kd_tiles, with_bo)
    if key not in _GRAPH_CACHE:
        _GRAPH_CACHE[key] = _build(kd_tiles, with_bo)
    return _GRAPH_CACHE[key]


def _pmajor(a, kd_tiles):
    """[kd_tiles*128, n] -> [128, kd_tiles, n] partition-major, contiguous."""
    n = a.shape[1]
    return np.ascontiguousarray(
        a.reshape(kd_tiles, 128, n).transpose(1, 0, 2))


def _make_in_maps(x, c, Wq, bq, Wk, bk, Wv, bv, Wo, bo):
    x = np.asarray(x, np.float32)
    c = np.asarray(c, np.float32)
    has_bias = any(np.any(np.asarray(b)) for b in (bq, bk, bv))
    with_bo = bool(np.any(np.asarray(bo)))
    kd_tiles = DT + (1 if has_bias else 0)
    KD = kd_tiles * 128

    def aug_w(W, b):
        wT = np.asarray(W, np.float32).T          # [D, D] feature-major
        if has_bias:
            pad = np.zeros((KD - D, D), np.float32)
            pad[0, :] = np.asarray(b, np.float32)
            wT = np.concatenate([wT, pad], axis=0)
        return _pmajor(wT.astype(BF), kd_tiles)

    wq_h = aug_w(Wq, bq)
    wk_h = aug_w(Wk, bk)
    wv_h = aug_w(Wv, bv)
    wo_h = _pmajor(np.ascontiguousarray(np.asarray(Wo, np.float32).T).astype(BF),
                   DT)

    def aug_act(aT):
        if has_bias:
            pad = np.zeros((KD - D, aT.shape[1]), np.float32)
            pad[0, :] = 1.0
            aT = np.concatenate([aT, pad], axis=0)
        return _pmajor(aT.astype(BF), kd_tiles)

    in_maps = []
    for core in range(N_CORES):
        b = core // (N_CORES // B)
        q0 = (core % (N_CORES // B)) * QL
        k0 = 2 * q0
        xs = x[b, q0:q0 + QL]                      # [QL, D]
        cs = c[b, k0:k0 + KL]                      # [KL, D]
        cperm = np.concatenate([cs[0::2], cs[1::2]], axis=0)
        m = {
            "xT": aug_act(np.ascontiguousarray(xs.T)),
            "cT": aug_act(np.ascontiguousarray(cperm.T)),
            "wq": wq_h,
            "wk": wk_h,
            "wv": wv_h,
            "wo": wo_h,
        }
        if with_bo:
            m["bo"] = np.asarray(bo, np.float32).reshape(1, D)
        in_maps.append(m)
    return in_maps, kd_tiles, with_bo


def _gather(results):
    out = np.empty((B, SQ, D), np.float32)
    for core in range(N_CORES):
        b = core // (N_CORES // B)
        q0 = (core % (N_CORES // B)) * QL
        # device layout [128, QT, D] -> rows q = qt*128 + p
        arr = results[core]["out"]
        out[b, q0:q0 + QL] = arr.transpose(1, 0, 2).reshape(QL, D)
    return out


def kernel(**inputs) -> np.ndarray:
    in_maps, kd_tiles, with_bo = _make_in_maps(**inputs)
    nc = _get_graph(kd_tiles, with_bo)
    res = run_bass_kernel_spmd(nc, in_maps, core_ids=list(range(N_CORES)))
    return _gather(res.results)


def run_traced(**inputs):
    """Like kernel() but with neuron-profile tracing; returns (out, results)."""
    in_maps, kd_tiles, with_bo = _make_in_maps(**inputs)
    nc = _get_graph(kd_tiles, with_bo)
    res = run_bass_kernel_spmd(nc, in_maps, core_ids=list(range(N_CORES)),
                               trace=True)
    return _gather(res.results), res
